# revision 1
# baseline (speedup 1.0000x reference)
"""Trainium2 Bass kernel for AdaptiveStochasticSNN (v2: fp16 GEMM, resident W1).

Model: x[B,T,D] -> FC1(D->H) -> StochasticAdaptiveLIF -> FC2(H->A)
       -> StochasticAdaptiveLIF -> mean spikes over T.   B,T,D,H,A = 256,64,6400,1000,4

Strategy (8 NeuronCores, data-parallel over batch, 32 batches/core):
- FC1 runs in fp16 (10-bit mantissa == TF32 precision for this data range;
  measured rel err identical to f32r at 0.0122) at full PE rate. W1 in fp16 is
  13.1MB -> fully SBUF-resident, eliminating the 39MB/core of W streaming the
  f32r version needed. x in fp16 halves streaming traffic to 26MB/core.
- Host layouts make every DMA contiguous per partition (5-10KB lines).
- The bernoulli draw  u < sigmoid(mem - 1 - theta)  is monotone-transformed on
  the host to  logit(u) + 1 + theta < mem  (no sigmoid on device); theta is
  tracked as psi = 20*theta - 10 so its update fuses to one DVE op.
- Time is processed in 5 windows of [16,16,16,12,4] timesteps (x BC=32 batch
  cols). GEMM(w) overlaps LIF1(w-1) + LIF2(w-1) on the VectorEngine; the small
  last window keeps the un-hidden tail recurrence short.
- psum->sbuf copies (fused +b1 on ACT) are interleaved with the last
  contraction chunk's matmuls so the next window's psum banks free early.
"""

import sys

sys.path.insert(0, "/opt/trn_rl_repo")

import numpy as np

# ---- problem dims (hardcoded; kernel.py must be self-contained) ----
B, T, D, H, A = 256, 64, 6400, 1000, 4
HP = 1024          # H padded to 8*128
NCORES = 8
BC = B // NCORES   # 32 batches per core
BT = BC * T        # 2048 bt-columns per core, ordered bt = t*BC + b
KC = D // 128      # 50 contraction chunks
MC = HP // 128     # 8 h-chunks
WSTEPS = [16, 16, 16, 8, 8]       # timesteps per window
NWIN = len(WSTEPS)
WCOLS = [s * BC for s in WSTEPS]  # 512,512,512,384,128
WCOL0 = [0]
for _c in WCOLS:
    WCOL0.append(WCOL0[-1] + _c)
WSTEP0 = [0]
for _s in WSTEPS:
    WSTEP0.append(WSTEP0[-1] + _s)
XG = 5             # kc chunks per x/w1 DMA group
NG = KC // XG      # 10 groups
NQ = BT // 128     # 16 lu1 column blocks (4 timesteps each)
BETA = 0.9
TH_DEC = 0.9
TH_PLUS = 0.05

_CACHE = {}


def _build_graph():
    import concourse.bass as bass
    import concourse.tile as tile
    from concourse import bacc, mybir
    from concourse.alu_op_type import AluOpType as op
    from contextlib import ExitStack

    F32 = mybir.dt.float32
    F16 = mybir.dt.float16
    AF = mybir.ActivationFunctionType

    nc = bacc.Bacc("TRN2", target_bir_lowering=False, debug=False, num_devices=NCORES)

    # host-prepped layouts, all contiguous per partition:
    # xh:  [p, w, kc, col]   fp16 (col within window)
    # w1h: [p, kc, mc, m]    fp16
    # lu1: [p, q, mc, c]     f32  (q = 128-col block = 4 timesteps)
    xh = nc.declare_dram_parameter("xh", [128, KC * BT], F16, isOutput=False)
    w1h = nc.declare_dram_parameter("w1h", [128, KC * MC * 128], F16, isOutput=False)
    b1t = nc.declare_dram_parameter("b1t", [128, MC], F32, isOutput=False)
    lu1 = nc.declare_dram_parameter("lu1", [128, NQ, MC, 128], F32, isOutput=False)
    lu2 = nc.declare_dram_parameter("lu2", [A, T * BC], F32, isOutput=False)
    w2t = nc.declare_dram_parameter("w2t", [128, MC, A], F16, isOutput=False)
    rs2b = nc.declare_dram_parameter("rs2b", [A, 512], F32, isOutput=False)
    out = nc.declare_dram_parameter("out", [A, BC], F32, isOutput=True)

    with tile.TileContext(nc) as tc, ExitStack() as ctx:
        p_w1 = ctx.enter_context(tc.tile_pool(name="w1p", bufs=1))
        p_x = ctx.enter_context(tc.tile_pool(name="xp", bufs=3))
        p_cur = ctx.enter_context(tc.tile_pool(name="curp", bufs=2))
        p_lu = ctx.enter_context(tc.tile_pool(name="lup", bufs=3))
        p_ge = ctx.enter_context(tc.tile_pool(name="gep", bufs=2))
        p_st = ctx.enter_context(tc.tile_pool(name="stp", bufs=1))
        p_sc = ctx.enter_context(tc.tile_pool(name="scp", bufs=2))
        p_c2 = ctx.enter_context(tc.tile_pool(name="c2p", bufs=2))
        p_ps = ctx.enter_context(
            tc.tile_pool(name="psp", bufs=8, space=bass.MemorySpace.PSUM)
        )

        # ---- constants / states ----
        b1_sb = p_st.tile([128, MC], F32, name="b1_sb")
        nc.sync.dma_start(b1_sb[:], b1t[:])
        w2_sb = p_st.tile([128, MC, A], F16, name="w2_sb")
        nc.sync.dma_start(w2_sb[:], w2t[:])
        rs2_sb = p_st.tile([A, 512], F32, name="rs2_sb")
        nc.sync.dma_start(rs2_sb[:], rs2b[:])

        w1_sb = p_w1.tile([128, KC * MC * 128], F16, name="w1_sb")

        mem = p_st.tile([128, MC, BC], F32, name="mem")
        nc.gpsimd.memset(mem[:], 0.0)
        psi = p_st.tile([128, MC, BC], F32, name="psi")
        nc.gpsimd.memset(psi[:], -10.0)
        mem2 = p_st.tile([A, BC], F32, name="mem2")
        nc.gpsimd.memset(mem2[:], 0.0)
        psi2 = p_st.tile([A, BC], F32, name="psi2")
        nc.gpsimd.memset(psi2[:], -10.0)
        lu2_sb = p_st.tile([A, T * BC], F32, name="lu2_sb")
        nc.sync.dma_start(lu2_sb[:], lu2[:])
        ge2a = p_st.tile([A, BC, T], F16, name="ge2a")

        cur_tiles = [None] * NWIN
        ge_tiles = [None] * NWIN
        c2_tiles = [None] * NWIN

        def emit_lif1_window(w):
            """LIF1 recurrence for window w on DVE (+ lu1 DMAs)."""
            ncol = WCOLS[w]
            cur1 = cur_tiles[w]
            ge_t = p_ge.tile([128, MC, ncol], F16, tag="ge", name=f"ge_{w}")
            ge_tiles[w] = ge_t
            q0 = WCOL0[w] // 128
            for q in range(ncol // 128):
                lu_t = p_lu.tile([128, MC, 128], F32, tag="lu", name=f"lu_{w}_{q}")
                nc.sync.dma_start(lu_t[:], lu1[:, q0 + q, :, :])
                for s4 in range(4):
                    s = q * 4 + s4
                    c_sl = cur1[:, :, s * BC : (s + 1) * BC]
                    mi = p_sc.tile([128, MC, BC], F32, tag="mi", name=f"mi_{w}_{s}")
                    nc.vector.scalar_tensor_tensor(
                        mi[:], mem[:], BETA, c_sl, op0=op.mult, op1=op.add
                    )
                    lp = p_sc.tile([128, MC, BC], F32, tag="lp", name=f"lp_{w}_{s}")
                    lu_sl = lu_t[:, :, s4 * BC : (s4 + 1) * BC]
                    nc.vector.scalar_tensor_tensor(
                        lp[:], psi[:], TH_PLUS, lu_sl, op0=op.mult, op1=op.add
                    )
                    ge_sl = ge_t[:, :, s * BC : (s + 1) * BC]
                    nc.vector.tensor_tensor(ge_sl, mi[:], lp[:], op.is_le)
                    nc.vector.tensor_tensor(mem[:], mi[:], ge_sl, op.mult)
                    nc.vector.scalar_tensor_tensor(
                        psi[:], psi[:], TH_DEC, ge_sl, op0=op.mult, op1=op.subtract
                    )

        ps2_tiles = [None] * NWIN

        def emit_fc2_mm(w):
            """FC2 matmuls for window w: ps2[A, ncol] = W2 @ ge_w."""
            ncol = WCOLS[w]
            ge_t = ge_tiles[w]
            ps2 = p_ps.tile([A, 512], F32, tag="acc", name=f"ps2_{w}")
            ps2_tiles[w] = ps2
            for k2 in range(MC):
                nc.tensor.matmul(
                    ps2[:, :ncol],
                    w2_sb[:, k2, :],
                    ge_t[:, k2, :],
                    start=(k2 == 0),
                    stop=(k2 == MC - 1),
                )

        def emit_fc2_sub(w):
            """c2 = rs2 - ps2 (DVE; emitted separately so the tail's LIF1
            never queues behind a subtract that waits on a boundary psum)."""
            ncol = WCOLS[w]
            c2 = p_c2.tile([A, 512], F32, tag="c2", name=f"c2_{w}")
            nc.vector.tensor_tensor(
                c2[:, :ncol], rs2_sb[:, :ncol], ps2_tiles[w][:, :ncol], op.subtract
            )
            c2_tiles[w] = c2

        def emit_fc2(w):
            emit_fc2_mm(w)
            emit_fc2_sub(w)

        def emit_lif2_window(w, s0=0, s1=None):
            """LIF2 recurrence steps [s0, s1) for window w on DVE."""
            if s1 is None:
                s1 = WSTEPS[w]
            for s in range(s0, s1):
                t = WSTEP0[w] + s
                cur2 = c2_tiles[w][:, s * BC : (s + 1) * BC]
                m2i = p_sc.tile([A, BC], F32, tag="m2i", name=f"m2i_{t}")
                nc.vector.scalar_tensor_tensor(
                    m2i[:], mem2[:], BETA, cur2, op0=op.mult, op1=op.add
                )
                lp2 = p_sc.tile([A, BC], F32, tag="lp2", name=f"lp2_{t}")
                nc.vector.scalar_tensor_tensor(
                    lp2[:],
                    psi2[:],
                    TH_PLUS,
                    lu2_sb[:, t * BC : (t + 1) * BC],
                    op0=op.mult,
                    op1=op.add,
                )
                ge2_sl = ge2a[:, :, t]
                nc.vector.tensor_tensor(ge2_sl, m2i[:], lp2[:], op.is_le)
                nc.vector.tensor_tensor(mem2[:], m2i[:], ge2_sl, op.mult)
                nc.vector.scalar_tensor_tensor(
                    psi2[:], psi2[:], TH_DEC, ge2_sl, op0=op.mult, op1=op.subtract
                )

        for w in range(NWIN):
            ncol = WCOLS[w]
            cur1 = p_cur.tile([128, MC, ncol], F32, tag="cur", name=f"cur1_{w}")
            cur_tiles[w] = cur1
            if w == NWIN - 1:
                # emitted pre-loop so ge(w-1) exists when fc2_mm(w-1) is
                # emitted inside this window's GEMM; DVE queue order is
                # unchanged (these ops still run during this window's GEMM)
                emit_lif1_window(w - 1)
                emit_lif2_window(w - 2)
            # the last window runs as two half-passes of 4 h-chunks (only 4
            # psum banks active per pass), so FC2(w-1) can borrow a free
            # bank between the passes and LIF2(w-1) hides under this GEMM
            halves = [range(MC)] if w < NWIN - 1 else [range(4), range(4, MC)]
            for hi, mcs in enumerate(halves):
                accs = {
                    mc: p_ps.tile([128, 512], F32, tag="acc", name=f"acc_{w}_{mc}")
                    for mc in mcs
                }
                for g in range(NG):
                    xg = p_x.tile(
                        [128, XG * ncol], F16, tag="x", name=f"x_{w}_{hi}_{g}"
                    )
                    src = WCOL0[w] * KC + g * XG * ncol
                    if w == 0 and g == 0:
                        # per-kc DMAs: the first matmul waits on only ~0.4MB
                        for i in range(XG):
                            nc.sync.dma_start(
                                xg[:, i * ncol : (i + 1) * ncol],
                                xh[:, src + i * ncol : src + (i + 1) * ncol],
                            )
                            wk = i * MC * 128
                            nc.sync.dma_start(
                                w1_sb[:, wk : wk + MC * 128],
                                w1h[:, wk : wk + MC * 128],
                            )
                    else:
                        nc.sync.dma_start(xg[:], xh[:, src : src + XG * ncol])
                        if w == 0:
                            wsrc = g * XG * MC * 128
                            nc.sync.dma_start(
                                w1_sb[:, wsrc : wsrc + XG * MC * 128],
                                w1h[:, wsrc : wsrc + XG * MC * 128],
                            )
                    for i in range(XG):
                        kc = g * XG + i
                        for mc in mcs:
                            nc.tensor.matmul(
                                accs[mc][:, :ncol],
                                w1_sb[
                                    :,
                                    (kc * MC + mc) * 128 : (kc * MC + mc + 1) * 128,
                                ],
                                xg[:, i * ncol : (i + 1) * ncol],
                                start=(kc == 0),
                                stop=(kc == KC - 1),
                            )
                            if kc == KC - 1:
                                # psum -> sbuf (+b1) right behind the final
                                # matmul of this h-chunk, freeing its bank
                                nc.scalar.activation(
                                    cur1[:, mc, :],
                                    accs[mc][:, :ncol],
                                    AF.Identity,
                                    bias=b1_sb[:, mc : mc + 1],
                                    scale=1.0,
                                )
                if w == NWIN - 1 and hi == 0:
                    emit_fc2_mm(w - 1)
            # DVE during window w: LIF1(w-1) first (it gates FC2(w-1)),
            # then LIF2(w-2) whose c2 has been ready since the previous
            # boundary
            if w >= 1 and w < NWIN - 1:
                emit_lif1_window(w - 1)
                if w >= 2:
                    emit_lif2_window(w - 2)
                emit_fc2(w - 1)
            elif w == NWIN - 1:
                emit_fc2_sub(w - 1)
                emit_lif2_window(w - 1)

        # ---------- tail ----------
        emit_lif1_window(NWIN - 1)
        emit_fc2_mm(NWIN - 1)
        emit_fc2_sub(NWIN - 1)
        emit_lif2_window(NWIN - 1)

        sum2 = p_st.tile([A, BC], F32, name="sum2")
        nc.vector.tensor_reduce(sum2[:], ge2a[:], mybir.AxisListType.X, op.add)
        outf = p_st.tile([A, BC], F32, name="outf")
        nc.scalar.activation(outf[:], sum2[:], AF.Copy, bias=1.0, scale=-1.0 / T)
        nc.sync.dma_start(out[:], outf[:])

    nc.compile()
    return nc


def _host_prep(x, W1, b1, W2, b2, u1, u2):
    """Shard + lay out inputs for the 8 cores. Returns in_maps."""
    x = np.asarray(x, dtype=np.float32)
    W1 = np.asarray(W1, dtype=np.float32)
    b1 = np.asarray(b1, dtype=np.float32)
    W2 = np.asarray(W2, dtype=np.float32)
    b2 = np.asarray(b2, dtype=np.float32)

    BIG = np.float32(1e30)
    with np.errstate(divide="ignore"):
        u1d = np.asarray(u1, dtype=np.float64)
        lu1f = np.clip(np.log(u1d / (1.0 - u1d)) + 1.5, -1e30, 1e30).astype(np.float32)
        u2d = np.asarray(u2, dtype=np.float64)
        lu2f = np.clip(np.log(u2d / (1.0 - u2d)) + 1.5, -1e30, 1e30).astype(np.float32)

    W1TP = np.zeros((D, HP), np.float32)
    W1TP[:, :H] = W1.T
    w1h = np.ascontiguousarray(
        W1TP.reshape(KC, 128, MC, 128).transpose(1, 0, 2, 3).reshape(128, KC * MC * 128)
    ).astype(np.float16)

    b1p = np.zeros((HP,), np.float32)
    b1p[:H] = b1
    b1t = np.ascontiguousarray(b1p.reshape(MC, 128).T)  # [128, MC]

    W2f16 = W2.T.astype(np.float16)  # [H, A]
    W2TP = np.zeros((HP, A), np.float16)
    W2TP[:H, :] = W2f16
    w2t = np.ascontiguousarray(W2TP.reshape(MC, 128, A).transpose(1, 0, 2))

    rs2 = (W2f16.astype(np.float64).sum(axis=0) + b2).astype(np.float32)  # [A]
    rs2b = np.ascontiguousarray(np.repeat(rs2[:, None], 512, axis=1))  # [A, 512]

    in_maps = []
    for c in range(NCORES):
        bs, be = c * BC, (c + 1) * BC
        # x: [D, bt] -> [p, w, kc, col] fp16
        xt = x[bs:be].transpose(2, 1, 0).reshape(D, BT).astype(np.float16)
        arr = xt.reshape(KC, 128, BT).transpose(1, 0, 2)  # [p, kc, bt]
        xh_c = np.concatenate(
            [
                np.ascontiguousarray(arr[:, :, WCOL0[w] : WCOL0[w + 1]]).reshape(
                    128, -1
                )
                for w in range(NWIN)
            ],
            axis=1,
        )
        xh_c = np.ascontiguousarray(xh_c)
        # lu1: [p, q, mc, c]
        lu_c = np.full((T, BC, HP), BIG, np.float32)
        lu_c[:, :, :H] = lu1f[:, bs:be, :]
        lu_c = lu_c.transpose(2, 0, 1).reshape(HP, BT)  # [h, t*BC+b]
        lu_c = lu_c.reshape(MC, 128, BT).transpose(1, 0, 2)  # [p, mc, bt]
        lu_c = np.ascontiguousarray(
            lu_c.reshape(128, MC, NQ, 128).transpose(0, 2, 1, 3)
        )  # [p, q, mc, c]
        # lu2: [A, T*BC]
        lu2_c = np.ascontiguousarray(
            lu2f[:, bs:be, :].transpose(2, 0, 1).reshape(A, T * BC)
        )
        in_maps.append(
            {
                "xh": xh_c,
                "w1h": w1h,
                "b1t": b1t,
                "lu1": lu_c,
                "lu2": lu2_c,
                "w2t": w2t,
                "rs2b": rs2b,
            }
        )
    return in_maps


def run(inputs, trace=False):
    """Build (cached), run on 8 cores, gather. Returns (out, BassKernelResults)."""
    from concourse.bass_utils import run_bass_kernel_spmd

    if "nc" not in _CACHE:
        _CACHE["nc"] = _build_graph()
    nc = _CACHE["nc"]
    in_maps = _host_prep(**inputs)
    res = run_bass_kernel_spmd(nc, in_maps, core_ids=list(range(NCORES)), trace=trace)
    out = np.concatenate(
        [res.results[c]["out"].T for c in range(NCORES)], axis=0
    )
    return np.ascontiguousarray(out, dtype=np.float32), res


def kernel(**inputs) -> np.ndarray:
    out, _ = run(inputs, trace=False)
    return out

